# revision 1
# baseline (speedup 1.0000x reference)
"""Deformable Conv2D (DCNv2 forward) as a Bass/Tile kernel on 8 TRN2 NeuronCores.

Sharding: 8 cores = (batch n in 0..3) x (output-row half h in 0..1).
Each core computes out[n, :, h*32:(h+1)*32, :]  (Cout=256 x 2048 positions).

Per-core pipeline (all fp32):
  1. Device computes bilinear sample indices + weights from (offset, mask, base grid).
  2. SWDGE indirect DMA gathers, per (tap k, 128-position tile), two 2-pixel-wide
     row segments per position (the x-pair (ps, ps+1) around x0 for rows y0 and y1)
     from the pixel-major input image in DRAM -> SBUF [128 pos, 512].
     x-edge cases are handled by remapping the bilinear weights onto the two slots.
  3. TensorEngine "diag-matmul" fuses scale-by-weight + transpose + 4-way reduce:
     psum[c, pos] += v_slot.T @ diag(w_slot), accumulated over the 4 (pair,slot) combos.
  4. fp32 GEMM: out[o, pos] = sum_{k,c} filt[o,c,k] * col[c,k,pos], accumulated in PSUM.
"""

import sys

sys.path.insert(0, "/opt/trn_rl_repo")

import numpy as np
from contextlib import ExitStack

import concourse.bass as bass
import concourse.mybir as mybir
import concourse.tile as tile
from concourse import bacc
from concourse import bass_utils

P = 128
H = W = 64
C = 256
CO = 256
K = 9
HR = 32          # output rows per core
NPOS = HR * W    # 2048 positions per core
NT = 16          # 128-position tiles
NT4 = 4          # 512-position groups
F32 = mybir.dt.float32
I32 = mybir.dt.int32
AL = mybir.AluOpType
AF = mybir.ActivationFunctionType

_PROGRAM_CACHE = {}


def _emit_index_weight_math(nc, cp, offm_sb, wk, idxg):
    """Device-side bilinear index/weight computation.

    offm_sb: [P, K, NT, 5] fields (dy, dx, mask, base_y, base_x); partition p of
    tile t is output position t*128+p.
    Writes:
      wk   [P, K, NT, 4] f32  slot weights: (AB,s0), (AB,s1), (CD,s0), (CD,s1)
      idxg [P, K, NT, 2] i32  gather row index (pixel y*64+ps) for pairs AB, CD
    """
    v = nc.vector
    sh = [P, K, NT]

    def t(name, dtype=F32, shape=None):
        return cp.tile(shape or sh, dtype, name=name, tag=name)

    dy = offm_sb[:, :, :, 0]
    dx = offm_sb[:, :, :, 1]
    mm = offm_sb[:, :, :, 2]
    by = offm_sb[:, :, :, 3]
    bx = offm_sb[:, :, :, 4]

    res = {}
    for ax, (base, off) in {"y": (by, dy), "x": (bx, dx)}.items():
        yy = t(f"{ax}_yy")
        v.tensor_tensor(yy[:], base, off, AL.add)
        yi = t(f"{ax}_yi", I32)
        v.tensor_copy(yi[:], yy[:])          # f32 -> i32 (round)
        yr = t(f"{ax}_yr")
        v.tensor_copy(yr[:], yi[:])          # back to f32
        gt = t(f"{ax}_gt")
        v.tensor_tensor(gt[:], yr[:], yy[:], AL.is_gt)   # 1.0 where round went up
        fy = t(f"{ax}_fy")
        v.tensor_tensor(fy[:], yr[:], gt[:], AL.subtract)  # floor
        ly = t(f"{ax}_ly")
        v.tensor_tensor(ly[:], yy[:], fy[:], AL.subtract)  # frac in [0,1)
        oly = t(f"{ax}_oly")
        v.tensor_scalar(oly[:], ly[:], -1.0, 1.0, AL.mult, AL.add)  # 1 - frac
        # clamped indices + validity for the two neighbors along this axis
        c0 = t(f"{ax}_c0")
        v.tensor_scalar(c0[:], fy[:], 0.0, 63.0, AL.max, AL.min)
        v0 = t(f"{ax}_v0")
        v.tensor_tensor(v0[:], c0[:], fy[:], AL.is_equal)
        f1 = t(f"{ax}_f1")
        v.tensor_scalar(f1[:], fy[:], 1.0, None, AL.add)
        c1 = t(f"{ax}_c1")
        v.tensor_scalar(c1[:], f1[:], 0.0, 63.0, AL.max, AL.min)
        v1 = t(f"{ax}_v1")
        v.tensor_tensor(v1[:], c1[:], f1[:], AL.is_equal)
        res[ax] = dict(f=fy, l=ly, ol=oly, c0=c0, c1=c1, v0=v0, v1=v1)

    ry, rx = res["y"], res["x"]
    # weight factors per axis (validity folded in); mask folded into y factors
    wy0 = t("wy0")
    v.tensor_tensor(wy0[:], ry["ol"][:], ry["v0"][:], AL.mult)
    v.tensor_tensor(wy0[:], wy0[:], mm, AL.mult)
    wy1 = t("wy1")
    v.tensor_tensor(wy1[:], ry["l"][:], ry["v1"][:], AL.mult)
    v.tensor_tensor(wy1[:], wy1[:], mm, AL.mult)
    wx0 = t("wx0")
    v.tensor_tensor(wx0[:], rx["ol"][:], rx["v0"][:], AL.mult)
    wx1 = t("wx1")
    v.tensor_tensor(wx1[:], rx["l"][:], rx["v1"][:], AL.mult)

    # raw bilinear weights: A=(y0,x0) B=(y0,x1) C=(y1,x0) D=(y1,x1)
    wA = t("wA"); v.tensor_tensor(wA[:], wy0[:], wx0[:], AL.mult)
    wB = t("wB"); v.tensor_tensor(wB[:], wy0[:], wx1[:], AL.mult)
    wC = t("wC"); v.tensor_tensor(wC[:], wy1[:], wx0[:], AL.mult)
    wD = t("wD"); v.tensor_tensor(wD[:], wy1[:], wx1[:], AL.mult)

    # x-pair start ps = clip(x0, 0, 62); slot0 = column ps, slot1 = column ps+1.
    # d0: x0 == ps (normal); dm: x0 == -1 (x1 lands on slot0); dp: x0 == 63 (x0 on slot1)
    fx = rx["f"]
    ps = t("ps")
    v.tensor_scalar(ps[:], fx[:], 0.0, 62.0, AL.max, AL.min)
    d0 = t("d0")
    v.tensor_tensor(d0[:], ps[:], fx[:], AL.is_equal)
    dm = t("dm")
    v.tensor_scalar(dm[:], fx[:], -1.0, None, AL.is_equal)
    dp = t("dp")
    v.tensor_scalar(dp[:], fx[:], 63.0, None, AL.is_equal)

    # slot weights: wk[...,0] = d0*wA + dm*wB ; wk[...,1] = d0*wB + dp*wA
    #               wk[...,2] = d0*wC + dm*wD ; wk[...,3] = d0*wD + dp*wC
    tmp1 = t("tmp1")
    tmp2 = t("tmp2")
    for qi, (sel1, w1, sel2, w2) in enumerate(
        [(d0, wA, dm, wB), (d0, wB, dp, wA), (d0, wC, dm, wD), (d0, wD, dp, wC)]
    ):
        v.tensor_tensor(tmp1[:], sel1[:], w1[:], AL.mult)
        v.tensor_tensor(tmp2[:], sel2[:], w2[:], AL.mult)
        v.tensor_tensor(wk[:, :, :, qi], tmp1[:], tmp2[:], AL.add)

    # gather row indices: AB pair at y0c*64 + ps, CD pair at y1c*64 + ps
    y0s = t("y0s")
    v.tensor_scalar(y0s[:], ry["c0"][:], 64.0, None, AL.mult)
    y1s = t("y1s")
    v.tensor_scalar(y1s[:], ry["c1"][:], 64.0, None, AL.mult)
    idxf = t("idxf", shape=[P, K, NT, 2])
    v.tensor_tensor(idxf[:, :, :, 0], y0s[:], ps[:], AL.add)
    v.tensor_tensor(idxf[:, :, :, 1], y1s[:], ps[:], AL.add)
    v.tensor_copy(idxg[:], idxf[:])


def _build_program(iters=1, static_gather=False):
    key = ("v2", iters, static_gather)
    if key in _PROGRAM_CACHE:
        return _PROGRAM_CACHE[key]

    nc = bacc.Bacc(
        "TRN2",
        target_bir_lowering=False,
        debug=False,
        enable_asserts=False,
        num_devices=8,
    )
    inp_pm = nc.dram_tensor("inp_pm", [H * W, C], F32, kind="ExternalInput")
    filt_d = nc.dram_tensor("filt", [P, K * 2 * 2 * P], F32, kind="ExternalInput")
    offm_d = nc.dram_tensor("offm", [P, K * NT * 5], F32, kind="ExternalInput")
    out_d = nc.dram_tensor("outp", [P, 2, NPOS], F32, kind="ExternalOutput")

    with tile.TileContext(nc) as tc, ExitStack() as ctx:
        cp = ctx.enter_context(tc.tile_pool(name="const", bufs=1))
        pv = ctx.enter_context(tc.tile_pool(name="v4", bufs=3))
        pd = ctx.enter_context(tc.tile_pool(name="diag", bufs=3))
        pcol = ctx.enter_context(tc.tile_pool(name="col", bufs=2))
        pout = ctx.enter_context(tc.tile_pool(name="osb", bufs=2))
        pps_col = ctx.enter_context(tc.tile_pool(name="pscol", bufs=3, space="PSUM"))
        pps_out = ctx.enter_context(tc.tile_pool(name="psout", bufs=1, space="PSUM"))

        filt_sb = cp.tile([P, K, 2, 2, P], F32, name="filt_sb")
        nc.sync.dma_start(
            filt_sb[:], filt_d.ap().rearrange("p (k c o j) -> p k c o j", k=K, c=2, o=2)
        )
        offm_sb = cp.tile([P, K, NT, 5], F32, name="offm_sb")
        nc.sync.dma_start(
            offm_sb[:], offm_d.ap().rearrange("p (k t f) -> p k t f", k=K, t=NT)
        )

        wk = cp.tile([P, K, NT, 4], F32, name="wk")
        idxg = cp.tile([P, K, NT, 2], I32, name="idxg")
        _emit_index_weight_math(nc, cp, offm_sb, wk, idxg)

        # constant stacked identity [P, 4, 128]: i4[p, q, j] = (p == j)
        i4 = cp.tile([P, 4, P], F32, name="i4")
        nc.gpsimd.memset(i4[:], 0.0)
        nc.gpsimd.affine_select(
            out=i4[:],
            in_=i4[:],
            pattern=[[0, 4], [-1, P]],
            compare_op=AL.not_equal,
            fill=1.0,
            base=0,
            channel_multiplier=1,
        )

        inp_ap = inp_pm.ap()

        for it in range(iters):
            for t4 in range(NT4):
                ps_out = pps_out.tile([P, 2, 512], F32, name="ps_out", tag="ps_out")
                col = pcol.tile([P, K, 2, 512], F32, name="col", tag="col")
                for k in range(K):
                    # gather: per (toff, pair) one indirect DMA of [128 pos, 512]
                    v4 = pv.tile([P, 4, 2, 512], F32, name="v4", tag="v4")
                    for tf in range(4):
                        tt = t4 * 4 + tf
                        for pair in range(2):
                            if static_gather:
                                nc.sync.dma_start(
                                    v4[:, tf, pair, :],
                                    inp_ap.rearrange("(a b) c -> a (b c)", b=2)[
                                        tf * 128 : (tf + 1) * 128, :
                                    ],
                                )
                            else:
                                nc.gpsimd.indirect_dma_start(
                                    out=v4[:, tf, pair, :],
                                    out_offset=None,
                                    in_=inp_ap[:],
                                    in_offset=bass.IndirectOffsetOnAxis(
                                        ap=idxg[:, k, tt, pair : pair + 1], axis=0
                                    ),
                                )
                    # diag weights [P, 4toff, 4slot, 128]
                    dg4 = pd.tile([P, 4, 4, P], F32, name="dg4", tag="dg4")
                    nc.vector.tensor_tensor(
                        dg4[:],
                        i4[:, None, :, :].to_broadcast((P, 4, 4, P)),
                        wk[:, k, t4 * 4 : (t4 + 1) * 4, :, None].to_broadcast(
                            (P, 4, 4, P)
                        ),
                        AL.mult,
                    )
                    pcolk = pps_col.tile([P, 2, 512], F32, name="pcolk", tag="pcolk")
                    for cc in range(2):
                        for tf in range(4):
                            for q in range(4):
                                # q = (pair, slot): value columns slot*256+cc*128
                                pair, slot = q // 2, q % 2
                                nc.tensor.matmul(
                                    pcolk[:, cc, tf * 128 : (tf + 1) * 128],
                                    lhsT=v4[
                                        :,
                                        tf,
                                        pair,
                                        slot * 256 + cc * 128 : slot * 256
                                        + (cc + 1) * 128,
                                    ],
                                    rhs=dg4[:, tf, q, :],
                                    start=(q == 0),
                                    stop=(q == 3),
                                )
                    nc.scalar.activation(col[:, k, 0], pcolk[:, 0], AF.Copy)
                    nc.vector.tensor_copy(col[:, k, 1], pcolk[:, 1])
                    for oc in range(2):
                        for cc in range(2):
                            nc.tensor.matmul(
                                ps_out[:, oc],
                                lhsT=filt_sb[:, k, cc, oc, :],
                                rhs=col[:, k, cc, :],
                                start=(k == 0 and cc == 0),
                                stop=(k == K - 1 and cc == 1),
                            )
                osb = pout.tile([P, 2, 512], F32, name="osb", tag="osb")
                nc.scalar.activation(osb[:, 0], ps_out[:, 0], AF.Copy)
                nc.vector.tensor_copy(osb[:, 1], ps_out[:, 1])
                nc.sync.dma_start(out_d.ap()[:, :, t4 * 512 : (t4 + 1) * 512], osb[:])

    nc.compile()
    # Strip sim-only trap/callback instructions before the NEFF build —
    # shipping them to hardware wedges the exec unit.
    from concourse.bass_interp import get_hw_module

    nc.m = get_hw_module(nc.m)
    _PROGRAM_CACHE[key] = nc
    return nc


def _pack_filter(filt):
    # filt [256, 256, 3, 3] -> [p(c_lo), k, chi, oc, j(o_lo)] -> [128, 4608]
    Wm = filt.reshape(CO, C, K)                       # [o, c, k]
    T = Wm.transpose(1, 2, 0).reshape(2, P, K, 2, P)  # [chi, p, k, oc, j]
    return np.ascontiguousarray(T.transpose(1, 2, 0, 3, 4).reshape(P, K * 2 * 2 * P))


def _pack_offm(offset_n, mask_n, h):
    # offset_n [18, 64, 64], mask_n [9, 64, 64] -> [128, K*NT*5]
    rows = slice(h * HR, (h + 1) * HR)
    pos = np.arange(NPOS)
    oy_g = (h * HR + pos // W).astype(np.float32)
    ox = (pos % W).astype(np.float32)
    A = np.empty((K, NPOS, 5), np.float32)
    for k in range(K):
        ki, kj = k // 3, k % 3
        A[k, :, 0] = offset_n[2 * k, rows].ravel()
        A[k, :, 1] = offset_n[2 * k + 1, rows].ravel()
        A[k, :, 2] = mask_n[k, rows].ravel()
        A[k, :, 3] = oy_g - 1.0 + ki
        A[k, :, 4] = ox - 1.0 + kj
    # [k, t*128+p, f] -> [p, k, t, f]
    return np.ascontiguousarray(
        A.reshape(K, NT, P, 5).transpose(2, 0, 1, 3).reshape(P, K * NT * 5)
    )


def make_in_maps(inputs, filter, offset, mask):
    inputs = np.ascontiguousarray(np.asarray(inputs, np.float32))
    filter = np.ascontiguousarray(np.asarray(filter, np.float32))
    offset = np.ascontiguousarray(np.asarray(offset, np.float32))
    mask = np.ascontiguousarray(np.asarray(mask, np.float32))
    filt_host = _pack_filter(filter)
    in_maps = []
    for core in range(8):
        n, hh = core // 2, core % 2
        ipm = np.ascontiguousarray(inputs[n].transpose(1, 2, 0).reshape(H * W, C))
        in_maps.append(
            {
                "inp_pm": ipm,
                "filt": filt_host,
                "offm": _pack_offm(offset[n], mask[n], hh),
            }
        )
    return in_maps


def assemble_output(results):
    out = np.zeros((4, CO, H, W), np.float32)
    for core in range(8):
        n, hh = core // 2, core % 2
        r = np.asarray(results[core]["outp"])  # [128 j, 2 oc, 2048 pos]
        r = r.reshape(P, 2, HR, W).transpose(1, 0, 2, 3).reshape(CO, HR, W)
        out[n][:, hh * HR : (hh + 1) * HR, :] = r
    return out


def kernel(inputs, filter, offset, mask):
    nc = _build_program()
    in_maps = make_in_maps(inputs, filter, offset, mask)
    res = bass_utils.run_bass_kernel_spmd(nc, in_maps, core_ids=list(range(8)))
    return assemble_output(res.results)

